# revision 12
# baseline (speedup 1.0000x reference)
"""Trainium2 Bass kernel for nn_JSONTreeLSTM (K=8192, L=128, D=64) on 8 NeuronCores.

Strategy
--------
Data-parallel over K: each core gets 1024 rows of x. The NumberEmbedder is
rank-1 (emb = x*w + b), so the LSTM input projection folds into 2 extra
contraction rows (u = W_ih_h@w and a ones-row carrying the total bias) of the
recurrent matmul. Per core the scan runs as 2 independent k-groups of width
512 (software-pipelined so TensorE/ScalarE/VectorE overlap across groups):

  per step t, per group:
    PG[:, 0:512]   = LA.T @ rhs     (gates [i; f], fp32, PSUM bank 0)
    PG[:, 512:1024]= LB.T @ rhs     (gates [o; 2g], PSUM bank 1)
    SAB = sigmoid(PG)               (one ACT op [128, 1024]; sigma(2g) = G)
    tg  = 2*G - 1                   (= tanh(g); DVE tensor_scalar)
    P   = [si; sf] * [tg; C]        (DVE tensor_tensor, bf16)
    C   = P_top + P_bot             (DVE add -> cell state)
    Th  = tanh(C)                   (ACT)
    h2  = so * Th                   (DVE -> fp32 rows 0:64 of next rhs)

The object-level reduction needs only per-core partial sums sum_k(h_L) and
sum_k(sigmoid(f)*c); f is computed with host-composed weights
(W_fh@W_aout). The tiny [1,64] object LSTM tail runs on host in float64.
"""

import os
import sys

import numpy as np

sys.path.insert(0, "/opt/trn_rl_repo")

import concourse.bass as bass
import concourse.mybir as mybir
import concourse.tile as tile
from concourse import bacc, bass_utils

K, L, D = 8192, 128, 64
NCORES = 8
KSH = K // NCORES      # 1024 rows per core
NG = 2                 # k-groups per core
W = KSH // NG          # 512
F32 = mybir.dt.float32
VEC_DT = mybir.dt.bfloat16
AF = mybir.ActivationFunctionType
ALU = mybir.AluOpType

_CACHE: dict = {}


def _sigmoid(z):
    return 1.0 / (1.0 + np.exp(-z))


def _prep_weights(inp):
    """Compose the device weight tiles (float64 math, cast to fp32)."""
    f = {k: np.asarray(v, np.float64) for k, v in inp.items()}
    W_ih_h = f["W_ih"][:, :D]                       # [256, 64]
    u = W_ih_h @ f["W_num"][:, 0]                   # [256]
    bias = f["b_ih"] + f["b_hh"] + W_ih_h @ f["b_num"]
    W_hh = f["W_hh"]                                # [256, 64]; rows i,f,g,o
    LA = W_hh[0:128].T                              # [64, 128]  [i; f]
    LB = np.concatenate([W_hh[192:256].T, 2.0 * W_hh[128:192].T], 1)  # [o; 2g]
    UX = np.stack([u[0:128],
                   np.concatenate([u[192:256], 2.0 * u[128:192]])])   # [2, 128]
    Wcomb = f["W_fh"] @ f["W_aout"]
    bias_f = f["W_fh"] @ f["b_aout"] + f["b_fh"]
    LF = Wcomb.T                                    # [64, 64]
    BIAS = np.zeros((128, 4))
    BIAS[:, 0] = bias[0:128]
    BIAS[:, 1] = np.concatenate([bias[192:256], 2.0 * bias[128:192]])
    BIAS[0:64, 2] = bias_f
    c = np.ascontiguousarray
    return (c(LA, np.float32), c(LB, np.float32), c(UX, np.float32),
            c(LF, np.float32), c(BIAS, np.float32))


def _build_nc(n_steps=L):
    nc = bacc.Bacc("TRN2")
    xT_d = nc.dram_tensor("xT", [L, KSH], F32, kind="ExternalInput")
    LA_d = nc.dram_tensor("LA", [64, 128], F32, kind="ExternalInput")
    LB_d = nc.dram_tensor("LB", [64, 128], F32, kind="ExternalInput")
    UX_d = nc.dram_tensor("UX", [2, 128], F32, kind="ExternalInput")
    LF_d = nc.dram_tensor("LF", [64, 64], F32, kind="ExternalInput")
    BIAS_d = nc.dram_tensor("BIAS", [128, 4], F32, kind="ExternalInput")
    out_d = nc.dram_tensor("out", [64, 4], F32, kind="ExternalOutput")

    with tile.TileContext(nc) as tc:
        with (
            tc.tile_pool(name="singles", bufs=1) as singles,
            tc.tile_pool(name="sab", bufs=2) as sab_pool,
            tc.tile_pool(name="pp", bufs=2) as p_pool,
            tc.tile_pool(name="th", bufs=2) as th_pool,
            tc.tile_pool(name="fin", bufs=1) as fin_pool,
            tc.tile_pool(name="psum", bufs=2, space="PSUM") as psum_pool,
        ):
            la = singles.tile([64, 128], F32, tag="la")
            lb = singles.tile([64, 128], F32, tag="lb")
            uxa = singles.tile([1, 128], F32, tag="uxa")
            uxb = singles.tile([1, 128], F32, tag="uxb")
            lf = singles.tile([64, 64], F32, tag="lf")
            bias = singles.tile([128, 4], F32, tag="bias")
            nc.sync.dma_start(la, LA_d[:, :])
            nc.sync.dma_start(lb, LB_d[:, :])
            nc.sync.dma_start(uxa, UX_d[0:1, :])
            nc.sync.dma_start(uxb, UX_d[1:2, :])
            nc.sync.dma_start(lf, LF_d[:, :])
            nc.sync.dma_start(bias, BIAS_d[:, :])

            RH = []   # RH[g][i]: [64, W] fp32 h state (rhs of recurrent mm)
            TS = []   # TS[g]: [128, W] bf16; rows 64:128 = C state
            XB = []   # XB[g][i]: [1, W] fp32 x_t row buffers
            for g in range(NG):
                rhs_g = [singles.tile([64, W], F32, tag=f"rh{g}_{i}",
                                      name=f"rh{g}_{i}")
                         for i in range(3)]
                xb_g = [singles.tile([1, W], F32, tag=f"xb{g}_{i}",
                                     name=f"xb{g}_{i}")
                        for i in range(3)]
                T_g = singles.tile([128, W], VEC_DT, tag=f"T{g}", name=f"T{g}")
                nc.vector.memset(rhs_g[0][:, :], 0.0)
                nc.vector.memset(T_g[:, :], 0.0)
                RH.append(rhs_g)
                TS.append(T_g)
                XB.append(xb_g)

            for t in range(n_steps):
                for g in range(NG):
                    gs = g * W
                    rh_t = RH[g][t % 3]
                    rh_n = RH[g][(t + 1) % 3]
                    T = TS[g]
                    xrow = XB[g][t % 3]
                    nc.sync.dma_start(xrow, xT_d[t:t + 1, gs:gs + W])
                    pg = psum_pool.tile([128, 2 * W], F32, tag=f"pg{g}")
                    nc.tensor.matmul(pg[:, 0:W], la, rh_t, start=True, stop=False)
                    nc.tensor.matmul(pg[:, 0:W], uxa, xrow, start=False, stop=True)
                    nc.tensor.matmul(pg[:, W:2 * W], lb, rh_t, start=True, stop=False)
                    nc.tensor.matmul(pg[:, W:2 * W], uxb, xrow, start=False, stop=True)
                    sab = sab_pool.tile([128, 2 * W], VEC_DT, tag=f"sab{g}")
                    nc.scalar.activation(sab[:, 0:W], pg[:, 0:W], AF.Sigmoid,
                                         bias=bias[:, 0:1])
                    nc.scalar.activation(sab[:, W:2 * W], pg[:, W:2 * W],
                                         AF.Sigmoid, bias=bias[:, 1:2])
                    # tg = 2*sigmoid(2g) - 1 = tanh(g)  (out at base partition 0)
                    tg = th_pool.tile([64, W], VEC_DT, tag=f"tg{g}", name=f"tg{g}")
                    nc.vector.tensor_scalar(
                        tg, sab[64:128, W:2 * W], 2.0, 1.0,
                        ALU.mult, ALU.subtract)
                    # pa = si*tg (bases 0/0); pb = sf*C (bases 64/64, out base 0)
                    pa = p_pool.tile([64, W], VEC_DT, tag=f"pa{g}", name=f"pa{g}")
                    pb = p_pool.tile([64, W], VEC_DT, tag=f"pb{g}", name=f"pb{g}")
                    nc.vector.tensor_mul(pa, sab[0:64, 0:W], tg)
                    nc.vector.tensor_mul(pb, sab[64:128, 0:W], T[64:128, :])
                    # C update lives at base 64 of T
                    nc.vector.tensor_add(T[64:128, :], pa, pb)
                    th = th_pool.tile([64, W], VEC_DT, tag=f"th{g}")
                    nc.scalar.activation(th, T[64:128, :], AF.Tanh)
                    nc.vector.tensor_mul(rh_n[:, :], sab[0:64, W:2 * W], th)

            # ---- final per-core partials ----
            for g in range(NG):
                rh_f = RH[g][n_steps % 3]
                pf = psum_pool.tile([64, W], F32, tag=f"pg{g}")
                nc.tensor.matmul(pf, lf, rh_f, start=True, stop=True)
                sf = fin_pool.tile([128, W], VEC_DT, tag=f"sf{g}")
                nc.scalar.activation(sf[64:128, :], pf, AF.Sigmoid,
                                     bias=bias[0:64, 2:3])
                scr = fin_pool.tile([64, W], VEC_DT, tag=f"scr{g}")
                fcs = fin_pool.tile([64, 1], F32, tag=f"fcs{g}")
                nc.vector.scalar_tensor_tensor(
                    scr, sf[64:128, :], 1.0, TS[g][64:128, :], ALU.mult, ALU.mult,
                    accum_out=fcs)
                hs = fin_pool.tile([64, 1], F32, tag=f"hs{g}")
                nc.vector.tensor_reduce(hs, rh_f[:, :],
                                        mybir.AxisListType.X, ALU.add)
                nc.sync.dma_start(out_d[:, g:g + 1], hs)
                nc.sync.dma_start(out_d[:, 2 + g:3 + g], fcs)

    nc.finalize()
    return nc


def _get_nc(n_steps=L):
    key = ("nc", n_steps)
    if key not in _CACHE:
        _CACHE[key] = _build_nc(n_steps)
    return _CACHE[key]


def _run_device(x, LA, LB, UX, LF, BIAS, trace=False, n_steps=L):
    nc = _get_nc(n_steps)
    in_maps = []
    for c in range(NCORES):
        xs = np.ascontiguousarray(x[c * KSH:(c + 1) * KSH].T, np.float32)
        in_maps.append({"xT": xs, "LA": LA, "LB": LB, "UX": UX,
                        "LF": LF, "BIAS": BIAS})
    import time
    t0 = time.time()
    res = bass_utils.run_bass_kernel_spmd(
        nc, in_maps, core_ids=list(range(NCORES)), trace=trace)
    _run_device.last_wall_s = time.time() - t0
    return res


def kernel(**inputs):
    inp = {k: np.asarray(v) for k, v in inputs.items()}
    LA, LB, UX, LF, BIAS = _prep_weights(inp)
    x = np.asarray(inp["x"], np.float32)
    trace = bool(int(os.environ.get("LSTM_TRACE", "0")))
    res = _run_device(x, LA, LB, UX, LF, BIAS, trace=trace)
    kernel._last_exec_ns = res.exec_time_ns
    hsum = np.zeros(64, np.float64)
    fcs = np.zeros(64, np.float64)
    for r in res.results:
        o = np.asarray(r["out"], np.float64)
        hsum += o[:, 0] + o[:, 1]
        fcs += o[:, 2] + o[:, 3]
    # ---- host: object-level TreeLSTM tail (tiny) ----
    f = {k: np.asarray(v, np.float64) for k, v in inp.items()}
    hs_bar = hsum @ f["W_aout"].T + K * f["b_aout"]
    iou = hs_bar @ f["W_iouh"].T + f["b_iouh"]
    i, o_, u = iou[0:64], iou[64:128], iou[128:192]
    c_obj = _sigmoid(i) * np.tanh(u) + fcs
    h_obj = _sigmoid(o_) * np.tanh(c_obj)
    h_hat = h_obj @ f["W_oout"].T + f["b_oout"]
    return np.concatenate([h_hat, c_obj])[None].astype(np.float32)


kernel._last_exec_ns = None
